# revision 19
# baseline (speedup 1.0000x reference)
"""TRN2 Bass kernel for nn_Adapter (dense_cnn): ViT adapter with two branches
  main:   h1 = xs@w1+b1 ; y = dwconv3d_3x3x3(h1)+cb ; y@w2+b2
  offset: g = xs@ow1    ; hoff = tdiff(g)+ob1 ; oc = dwconv_1x3x3(hoff)+ocb ; oc@ow2+ob2
  out = x with patch tokens += main + offset   (CLS rows pass through)

Data-parallel over 8 NeuronCores: 2 clips (16 frames) per core; adapter
weights replicated. Per-core kernel (raw bass, explicit semaphores,
fine-grained pipelining, fp8-e4m3 DoubleRow matmuls):
  - activations channel-major; x^T supplied pre-transposed (fp8) per shard
  - all three matmul stages run fp8 DoubleRow (2 contraction rows/cycle);
    depthwise convs are PSUM-accumulated diagonal DR matmuls on PE with
    taps paired within equal-dw groups (DR mid-step must be 16-aligned);
    conv outputs are scaled x16 (folded into tap weights, /16 in w2) to
    keep fp8 conv_out in the e4m3 sweet spot
  - a tunable subset of conv chunks runs on DVE (bf16 scratch accumulator,
    final cast to fp8) to balance engines
  - matmul2 token-major; DVE fuses the +x residual in f32 on evict

Self-contained: hardcodes shapes for x:[128,197,768], T=8 (asserts).
"""
import numpy as np
import ml_dtypes

import concourse.bass as bass
import concourse.mybir as mybir
from concourse.bass_utils import run_bass_kernel_spmd

F32 = mybir.dt.float32
BF16 = mybir.dt.bfloat16
F8 = mybir.dt.float8e4
AOT = mybir.AluOpType
AFT = mybir.ActivationFunctionType
DR = mybir.MatmulPerfMode.DoubleRow
BF = ml_dtypes.bfloat16
F8NP = ml_dtypes.float8_e4m3

# ---- problem constants (per core) ----
C = 768
CA = 384
T = 8
NPL = 256
CLIPS = 2
NPIX_CLIP = T * 14 * 14
NPIX = CLIPS * NPIX_CLIP
KC = C // 128
NG = CA // 128
H1PAD = NG * CLIPS * (T + 2) * NPL
GPAD = NG * CLIPS * T * NPL
GUARD = NPL
NTIL2 = (NPIX + 127) // 128
M1_CH = 392
OUT_ROWS = NPIX + 16
CVS = 16.0   # weight up-scale per stage; /CVS**3 folded into final evict

# tap (dt, dh, dw) lists grouped by dw so DR pairs share dw (step % 16 == 0)
def _pairs(taps):
    by_dw = {}
    for tp in taps:
        by_dw.setdefault(tp[2], []).append(tp)
    prs = []
    for dw in sorted(by_dw):
        grp = by_dw[dw]
        for i in range(0, len(grp) - 1, 2):
            prs.append((grp[i], grp[i + 1]))
        if len(grp) % 2:
            prs.append((grp[-1], None))
    return prs

MAIN_TAPS = [(kd - 1, kh - 1, kw - 1)
             for kd in range(3) for kh in range(3) for kw in range(3)]
OFF_TAPS = [(0, kh - 1, kw - 1) for kh in range(3) for kw in range(3)]
MAIN_PAIRS = _pairs(MAIN_TAPS)   # 15 (12 pairs + 3 singles)
OFF_PAIRS = _pairs(OFF_TAPS)     # 6 (3 pairs + 3 singles)
NPR_MAIN = len(MAIN_PAIRS)
NPR_OFF = len(OFF_PAIRS)
NPR_TOT = (NPR_MAIN + NPR_OFF) * NG   # 63

CONV_CHUNKS = [(br, g, c, tc)
               for c in range(CLIPS) for tc in range(4)
               for br in (0, 1) for g in range(NG)]


def _dve_set(n_main, n_off):
    mains = [ch for ch in CONV_CHUNKS if ch[0] == 1]
    offs = [ch for ch in CONV_CHUNKS if ch[0] == 0]
    pick = lambda lst, n: {lst[round(i * len(lst) / n) % len(lst)] for i in range(n)} if n else set()
    return pick(mains, n_main) | pick(offs, n_off)


DVE_CHUNKS = _dve_set(3, 0)


def build(debug=False, dve_chunks=None):
    dvec = DVE_CHUNKS if dve_chunks is None else dve_chunks
    nc = bass.Bass()
    xT = nc.declare_dram_parameter("xT", [C, NPIX], F8, isOutput=False)
    xtok = nc.declare_dram_parameter("xtok", [NPIX, C], F32, isOutput=False)
    xcls = nc.declare_dram_parameter("xcls", [16, C], F32, isOutput=False)
    w1c = nc.declare_dram_parameter("w1c", [128, KC // 2 * 2 * C], F8, isOutput=False)
    w2c = nc.declare_dram_parameter("w2c", [128, KC // 2 * 2 * C], F8, isOutput=False)
    diag = nc.declare_dram_parameter("diag", [128, NPR_TOT * 2 * 128], F8, isOutput=False)
    b1c = nc.declare_dram_parameter("b1c", [128, KC], F32, isOutput=False)
    cbc = nc.declare_dram_parameter("cbc", [128, KC], F32, isOutput=False)
    wtp = nc.declare_dram_parameter("wtp", [128, (27 + 9) * NG], F32, isOutput=False)
    out = nc.declare_dram_parameter("out", [OUT_ROWS, C], F32, isOutput=True)
    if debug:
        dbg_h1 = nc.declare_dram_parameter("dbg_h1", [128, H1PAD + 2 * GUARD], F8, isOutput=True)
        dbg_g = nc.declare_dram_parameter("dbg_g", [128, GPAD + 2 * GUARD], F8, isOutput=True)
        dbg_cv = nc.declare_dram_parameter("dbg_cv", [128, KC * NPIX], F8, isOutput=True)

    xT_sb = nc.alloc_sbuf_tensor([128, KC * NPIX], F8)
    w1_sb = nc.alloc_sbuf_tensor([128, KC // 2 * 2 * C], F8)   # [pair][s][m]
    w2_sb = nc.alloc_sbuf_tensor([128, KC // 2 * 2 * C], F8)
    diag_sb = nc.alloc_sbuf_tensor([128, NPR_TOT * 2 * 128], F8)  # [pr][s][m]
    b1_sb = nc.alloc_sbuf_tensor([128, KC], F32)
    cb_sb = nc.alloc_sbuf_tensor([128, KC], F32)
    wt_sb = nc.alloc_sbuf_tensor([128, (27 + 9) * NG], F32)
    h1p = nc.alloc_sbuf_tensor([128, H1PAD + 2 * GUARD], F8)
    gp = nc.alloc_sbuf_tensor([128, GPAD + 2 * GUARD], F8)
    cvo = nc.alloc_sbuf_tensor([128, KC * NPIX], F8)
    scr = nc.alloc_sbuf_tensor([128, 2 * 196], BF16)    # DVE conv scratch (2 bufs)
    xtk = nc.alloc_sbuf_tensor([128, 3 * C], F32)
    ost = nc.alloc_sbuf_tensor([128, 2 * C], F32)
    ps = nc.alloc_psum_tensor([128, 4096], F32)

    M_ORDER = [3, 4, 5, 0, 1, 2]

    def h1_plane(g, c, tpad):
        return GUARD + ((g * CLIPS + c) * (T + 2) + tpad) * NPL

    def g_plane(g, c, t):
        return GUARD + ((g * CLIPS + c) * T + t) * NPL

    def sv3(buf, ext, offset, dims):
        """3D free view [part + dims] of an sbuf tensor via explicit AP."""
        return bass.AP(buf, offset, [[ext, 128]] + [list(d) for d in dims])

    XT_EXT = KC * NPIX
    W_EXT = KC // 2 * 2 * C
    DG_EXT = NPR_TOT * 2 * 128
    H1_EXT = H1PAD + 2 * GUARD
    GP_EXT = GPAD + 2 * GUARD
    CV_EXT = KC * NPIX

    # ---------- static schedules ----------
    m1_chunks = [(mi, m, j) for mi, m in enumerate(M_ORDER) for j in range(8)]
    conv_pe = [ch for ch in CONV_CHUNKS if ch not in dvec]
    conv_dve = [ch for ch in CONV_CHUNKS if ch in dvec]
    N_M1 = len(m1_chunks)
    N_CPE = len(conv_pe)
    N_CDVE = len(conv_dve)
    ACT_ALL = 2 * (N_M1 + N_CPE)
    DIFF_ALL = 2 + NG * CLIPS
    DVE_CONV_DONE = DIFF_ALL + N_CDVE
    DVE_EV = lambda i: DVE_CONV_DONE + i + 1
    LD_XT, LD_W1, LD_W2, LD_DIAG, LD_B1, LD_CB, LD_WT = (16 * i for i in range(1, 8))
    LD_XTOK = lambda i: 16 * (8 + i)

    def m1_thr(br, g, c, tc):
        mi = (3 + g) if br else g
        jmax = c * 4 + min(tc + 1, 3) if br else c * 4 + 3
        return 2 * (mi * 8 + jmax) + 2

    def conv_counts_thru(c_hi, tc_hi):
        npe = ndve = 0
        for ch in CONV_CHUNKS:
            br, g, c, tc = ch
            if (c, tc) > (c_hi, tc_hi):
                continue
            if ch in dvec:
                ndve += 1
            else:
                npe += 1
        return npe, ndve

    M2_THR = []
    for i in range(NTIL2):
        p_hi = (min(128 * (i + 1), NPIX) - 1) // 196
        c_hi, t_hi = divmod(p_hi, T)
        npe, ndve = conv_counts_thru(c_hi, t_hi // 2)
        M2_THR.append((2 * (N_M1 + npe), DIFF_ALL + ndve))

    with (
        nc.Block() as block,
        nc.semaphore("s_ld") as s_ld,
        nc.semaphore("s_pe") as s_pe,
        nc.semaphore("s_act") as s_act,
        nc.semaphore("s_dve") as s_dve,
        nc.semaphore("s_out") as s_out,
        nc.semaphore("s_dbg") as s_dbg,
    ):
        # ================= SP: all DMA =================
        @block.sync
        def _(sync):
            sync.dma_start(out=out[NPIX:OUT_ROWS, :], in_=xcls[:]).then_inc(s_out, 16)
            sync.dma_start(
                out=xT_sb[:].rearrange("p (k n) -> p k n", k=KC),
                in_=xT[:].rearrange("(k p) n -> p k n", p=128),
            ).then_inc(s_ld, 16)
            sync.dma_start(out=w1_sb[:], in_=w1c[:]).then_inc(s_ld, 16)
            sync.dma_start(out=w2_sb[:], in_=w2c[:]).then_inc(s_ld, 16)
            sync.dma_start(out=diag_sb[:], in_=diag[:]).then_inc(s_ld, 16)
            sync.dma_start(out=b1_sb[:], in_=b1c[:]).then_inc(s_ld, 16)
            sync.dma_start(out=cb_sb[:], in_=cbc[:]).then_inc(s_ld, 16)
            sync.dma_start(out=wt_sb[:], in_=wtp[:]).then_inc(s_ld, 16)
            for i in range(3):
                rows = min(128, NPIX - i * 128)
                sync.dma_start(out=xtk[:rows, bass.ts(i, C)],
                               in_=xtok[i * 128:i * 128 + rows, :]).then_inc(s_ld, 16)
            if debug:
                sync.wait_ge(s_act, 2 * N_M1)
                sync.wait_ge(s_dve, DIFF_ALL)
                sync.dma_start(out=dbg_h1[:], in_=h1p[:]).then_inc(s_dbg, 16)
                sync.dma_start(out=dbg_g[:], in_=gp[:]).then_inc(s_dbg, 16)
                sync.wait_ge(s_act, ACT_ALL)
                sync.wait_ge(s_dve, DVE_CONV_DONE)
                sync.dma_start(out=dbg_cv[:], in_=cvo[:]).then_inc(s_dbg, 16)
            for i in range(NTIL2):
                rows = min(128, NPIX - i * 128)
                sync.wait_ge(s_dve, DVE_EV(i))
                sync.dma_start(out=out[i * 128:i * 128 + rows, :],
                               in_=ost[:rows, bass.ts(i % 2, C)]).then_inc(s_out, 16)
                j = i + 3
                if j < NTIL2:
                    rows2 = min(128, NPIX - j * 128)
                    sync.dma_start(out=xtk[:rows2, bass.ts(j % 3, C)],
                                   in_=xtok[j * 128:j * 128 + rows2, :]).then_inc(s_ld, 16)
            if debug:
                sync.wait_ge(s_dbg, 48)

        # ================= PE =================
        @block.tensor
        def _(tensor):
            tensor.wait_ge(s_ld, LD_W1)
            # ---- matmul1 (DR, 3 chunk-pairs), banks 0..7 rotating ----
            for q, (mi, m, j) in enumerate(m1_chunks):
                bank = q % 8
                if q >= 8:
                    tensor.wait_ge(s_act, 2 * (q - 8) + 2)
                pv = ps[:, bank * 512: bank * 512 + M1_CH]
                for pr in range(KC // 2):
                    lhsT = sv3(w1_sb, W_EXT, pr * 2 * C + m * 128,
                               [(C, 2), (1, 128)])
                    rhs = sv3(xT_sb, XT_EXT, (pr * 2) * NPIX + j * M1_CH,
                              [(NPIX, 2), (1, M1_CH)])
                    mm = tensor.matmul(pv, lhsT, rhs, perf_mode=DR,
                                       start=(pr == 0), stop=(pr == KC // 2 - 1))
                mm.then_inc(s_pe, 1)
            # ---- conv (PE chunks, DR pairs), banks 4..7 rotating ----
            tensor.wait_ge(s_ld, LD_DIAG)
            for qc, (br, g, c, tc) in enumerate(conv_pe):
                bank = 4 + qc % 4
                if qc >= 4:
                    tensor.wait_ge(s_act, 2 * (N_M1 + qc - 4) + 2)
                else:
                    tensor.wait_ge(s_act, 2 * (44 + qc) + 2)
                tensor.wait_ge(s_act, m1_thr(br, g, c, tc))
                if br == 0:
                    tensor.wait_ge(s_dve, 3 + g * 2 + c)
                pairs = MAIN_PAIRS if br else OFF_PAIRS
                prbase = 0 if br else NPR_MAIN * NG
                pv = ps[:, bank * 512:(bank + 1) * 512]
                for ip, (tA, tB) in enumerate(pairs):
                    dtA, dhA, dwA = tA
                    if br:
                        offA = h1_plane(g, c, 2 * tc + 1 + dtA) + dhA * 16 + dwA
                        buf, ext = h1p, H1_EXT
                    else:
                        offA = g_plane(g, c, 2 * tc + dtA) + dhA * 16 + dwA
                        buf, ext = gp, GP_EXT
                    if tB is None:
                        sstep = 16
                    else:
                        dtB, dhB, dwB = tB
                        sstep = (dtB - dtA) * 256 + (dhB - dhA) * 16
                    lhsT = sv3(diag_sb, DG_EXT, (prbase + ip * NG + g) * 256,
                               [(128, 2), (1, 128)])
                    rhs = sv3(buf, ext, offA, [(sstep, 2), (1, 512)])
                    mm = tensor.matmul(pv, lhsT, rhs, perf_mode=DR,
                                       start=(ip == 0), stop=(ip == len(pairs) - 1),
                                       skip_group_check=True)
                mm.then_inc(s_pe, 1)
            # ---- matmul2 (DR, group-pairs), psum pairs {0,1}/{2,3} ----
            for i in range(NTIL2):
                rows = min(128, NPIX - i * 128)
                ta, td = M2_THR[i]
                tensor.wait_ge(s_act, ta)
                tensor.wait_ge(s_dve, td)
                if i >= 2:
                    tensor.wait_ge(s_dve, DVE_EV(i - 2))
                pv = ps[:rows, (i % 2) * 1024:(i % 2) * 1024 + 768]
                for pr in range(KC // 2):
                    lhsT = sv3(cvo, CV_EXT, (pr * 2) * NPIX + i * 128,
                               [(NPIX, 2), (1, rows)])
                    tensor.matmul(pv[:, 0:512], lhsT,
                                  sv3(w2_sb, W_EXT, pr * 2 * C, [(C, 2), (1, 512)]),
                                  perf_mode=DR,
                                  start=(pr == 0), stop=(pr == KC // 2 - 1),
                                  skip_group_check=True)
                    mm1 = tensor.matmul(pv[:, 512:768], lhsT,
                                        sv3(w2_sb, W_EXT, pr * 2 * C + 512,
                                            [(C, 2), (1, 256)]),
                                        perf_mode=DR,
                                        start=(pr == 0), stop=(pr == KC // 2 - 1),
                                        skip_group_check=True)
                mm1.then_inc(s_pe, 1)

        # ================= ACT: psum evicts =================
        @block.scalar
        def _(scalar):
            scalar.wait_ge(s_ld, LD_CB)
            h1v = h1p[:, GUARD:GUARD + H1PAD].rearrange(
                "p (qq h w) -> p qq h w", h=16, w=16)
            gv = gp[:, GUARD:GUARD + GPAD].rearrange(
                "p (qq h w) -> p qq h w", h=16, w=16)
            seen_h1 = False
            for q, (mi, m, j) in enumerate(m1_chunks):
                bank = q % 8
                scalar.wait_ge(s_pe, q + 1)
                if q == 0:
                    scalar.wait_ge(s_dve, 1)
                if m < 3 and not seen_h1:
                    scalar.wait_ge(s_dve, 2)
                    seen_h1 = True
                for pl in range(2):
                    gt = 2 * j + pl
                    c, t = divmod(gt, T)
                    src = ps[:, bank * 512 + pl * 196: bank * 512 + (pl + 1) * 196
                             ].rearrange("p (h w) -> p h w", h=14)
                    if m < 3:
                        dst = h1v[:, (m * CLIPS + c) * (T + 2) + t + 1, 1:15, 1:15]
                        bias = b1_sb[:, m:m + 1]
                    else:
                        dst = gv[:, ((m - 3) * CLIPS + c) * T + t, 1:15, 1:15]
                        bias = 0.0
                    scalar.activation(dst, src, AFT.Identity,
                                      bias=bias).then_inc(s_act, 1)
            for qc, (br, g, c, tc) in enumerate(conv_pe):
                bank = 4 + qc % 4
                scalar.wait_ge(s_pe, N_M1 + qc + 1)
                grp = g if br else 3 + g
                for pl in range(2):
                    t = 2 * tc + pl
                    src = ps[:, bank * 512 + pl * NPL + 17:
                             bank * 512 + pl * NPL + 17 + 14 * 16
                             ].rearrange("p (h w) -> p h w", w=16)[:, :, 0:14]
                    dst = cvo[:, grp * NPIX + c * NPIX_CLIP + t * 196:
                              grp * NPIX + c * NPIX_CLIP + (t + 1) * 196
                              ].rearrange("p (h w) -> p h w", h=14)
                    scalar.activation(dst, src, AFT.Identity,
                                      bias=cb_sb[:, grp:grp + 1]).then_inc(s_act, 1)

        # ================= DVE =================
        @block.vector
        def _(vector):
            vector.memset(gp[:], 0.0).then_inc(s_dve, 1)
            vector.memset(h1p[:], 0.0).then_inc(s_dve, 1)
            vector.wait_ge(s_ld, LD_WT)
            for g in range(NG):
                for c in range(CLIPS):
                    vector.wait_ge(s_act, 2 * (g * 8 + c * 4 + 4))
                    for t in range(T - 1, 0, -1):
                        a = g_plane(g, c, t)
                        b = g_plane(g, c, t - 1)
                        last = vector.tensor_tensor(
                            gp[:, a:a + NPL], gp[:, a:a + NPL], gp[:, b:b + NPL],
                            op=AOT.subtract)
                    z = g_plane(g, c, 0)
                    last = vector.tensor_tensor(
                        gp[:, z:z + NPL], gp[:, z:z + NPL], gp[:, z:z + NPL],
                        op=AOT.subtract)
                    ob1 = b1_sb[:, 3 + g:4 + g]
                    for t in range(T):
                        base = g_plane(g, c, t)
                        iv = gp[:, base + 17: base + 17 + 14 * 16].rearrange(
                            "p (h w) -> p h w", w=16)[:, :, 0:14]
                        last = vector.tensor_scalar(iv, iv, ob1, None, op0=AOT.add)
                    last.then_inc(s_dve, 1)
            # ---- conv chunks on DVE (bf16 scratch acc, cast to fp8 at end) ----
            for br, g, c, tc in conv_dve:
                if br == 1:
                    vector.wait_ge(s_act, m1_thr(br, g, c, tc))
                taps = MAIN_TAPS if br else OFF_TAPS
                grp = g if br else 3 + g
                wbase = (0 if br else 27 * NG)
                for pl in range(2):
                    t = 2 * tc + pl
                    acc = scr[:, pl * 196:(pl + 1) * 196].rearrange(
                        "p (h w) -> p h w", h=14)
                    for it, (dt, dh, dw) in enumerate(taps):
                        if br:
                            base = h1_plane(g, c, t + 1 + dt)
                            srcbuf = h1p
                        else:
                            base = g_plane(g, c, t + dt)
                            srcbuf = gp
                        svv = srcbuf[:, base + 17 + dh * 16 + dw:
                                     base + 17 + dh * 16 + dw + 14 * 16
                                     ].rearrange("p (h w) -> p h w", w=16)[:, :, 0:14]
                        wsc = wt_sb[:, wbase + it * NG + g: wbase + it * NG + g + 1]
                        if it == 0:
                            vector.tensor_scalar(
                                acc, svv, wsc, cb_sb[:, grp:grp + 1],
                                op0=AOT.mult, op1=AOT.add)
                        else:
                            vector.scalar_tensor_tensor(
                                acc, svv, wsc, acc, op0=AOT.mult, op1=AOT.add)
                    dst = cvo[:, grp * NPIX + c * NPIX_CLIP + t * 196:
                              grp * NPIX + c * NPIX_CLIP + (t + 1) * 196
                              ].rearrange("p (h w) -> p h w", h=14)
                    last = vector.tensor_copy(dst, acc)
                last.then_inc(s_dve, 1)
            # ---- m2 evict + residual ----
            for i in range(NTIL2):
                rows = min(128, NPIX - i * 128)
                vector.wait_ge(s_pe, N_M1 + N_CPE + i + 1)
                vector.wait_ge(s_ld, LD_XTOK(i))
                if i >= 2:
                    vector.wait_ge(s_out, 16 * i)
                vector.scalar_tensor_tensor(
                    ost[:rows, bass.ts(i % 2, C)],
                    ps[:rows, (i % 2) * 1024:(i % 2) * 1024 + 768],
                    1.0 / (CVS ** 3),
                    xtk[:rows, bass.ts(i % 3, C)],
                    op0=AOT.mult, op1=AOT.add).then_inc(s_dve, 1)

    return nc


# ---------------- host side ----------------
_NC_CACHE = {}


def _get_nc():
    if "nc" not in _NC_CACHE:
        _NC_CACHE["nc"] = build()
    return _NC_CACHE["nc"]


def _dr_pack(W):
    """[768(k), M] -> per-partition DR layout [128(ki), pair, s, M] flattened."""
    M = W.shape[1]
    out = np.zeros((128, KC // 2, 2, M), np.float32)
    for pr in range(KC // 2):
        for s in range(2):
            out[:, pr, s, :] = W[pr * 256 + s * 128: pr * 256 + (s + 1) * 128, :]
    return out.reshape(128, KC // 2 * 2 * M)


def _prep_weights(w1, b1, cw, cb, w2, b2, ow1, ob1, ocw, ocb, ow2, ob2):
    w1c = _dr_pack(np.hstack([w1, ow1]) * CVS).astype(F8NP)
    w2c = _dr_pack(np.vstack([w2, ow2]) * CVS).astype(F8NP)
    # diag DR pairs: [128(ki), pr_tot, s, 128(m)] with diagonal per s
    diag = np.zeros((128, NPR_TOT, 2, 128), np.float32)
    wtp = np.zeros((128, (27 + 9) * NG), np.float32)
    eye = np.eye(128, dtype=bool)

    def tapw(w_, tp, main):
        dt, dh, dw = tp
        if main:
            return w_[:, 0, dt + 1, dh + 1, dw + 1]
        return w_[:, 0, 0, dh + 1, dw + 1]

    for br, (pairs, w_, base) in enumerate(
            [(MAIN_PAIRS, cw, 0), (OFF_PAIRS, ocw, NPR_MAIN * NG)]):
        for ip, (tA, tB) in enumerate(pairs):
            for g in range(NG):
                pi = base + ip * NG + g
                vA = tapw(w_, tA, br == 0) * CVS
                diag[:, pi, 0, :][eye] = vA[g * 128:(g + 1) * 128]
                if tB is not None:
                    vB = tapw(w_, tB, br == 0) * CVS
                    diag[:, pi, 1, :][eye] = vB[g * 128:(g + 1) * 128]
    i = 0
    for kd in range(3):
        for kh in range(3):
            for kw in range(3):
                for g in range(NG):
                    wtp[:, i] = cw[g * 128:(g + 1) * 128, 0, kd, kh, kw] * CVS
                    i += 1
    for kh in range(3):
        for kw in range(3):
            for g in range(NG):
                wtp[:, i] = ocw[g * 128:(g + 1) * 128, 0, 0, kh, kw] * CVS
                i += 1
    b1cv = np.ascontiguousarray(
        (np.concatenate([b1, ob1]) * CVS).reshape(KC, 128).T).astype(np.float32)
    cbcv = np.ascontiguousarray(
        (np.concatenate([cb, ocb]) * CVS * CVS).reshape(KC, 128).T).astype(np.float32)
    bias2 = (b2 + ob2).astype(np.float32)
    return dict(w1c=w1c, w2c=w2c,
                diag=diag.reshape(128, NPR_TOT * 2 * 128).astype(F8NP),
                b1c=b1cv, cbc=cbcv, wtp=wtp), bias2


def kernel(**inputs):
    x = np.asarray(inputs["x"], dtype=np.float32)
    Tv = int(np.asarray(inputs["T"]))
    assert Tv == T and x.shape == (128, 197, C)
    wd, bias2 = _prep_weights(
        *[np.asarray(inputs[k], dtype=np.float32) for k in
          ("w1", "b1", "cw", "cb", "w2", "b2", "ow1", "ob1", "ocw", "ocb", "ow2", "ob2")])

    in_maps = []
    for core in range(8):
        xs = x[core * 16:(core + 1) * 16]
        xpat = np.ascontiguousarray(xs[:, 1:, :]).reshape(NPIX, C)
        m = dict(wd)
        m["xT"] = np.ascontiguousarray(xpat.T).astype(F8NP)
        m["xtok"] = (xpat + bias2).astype(np.float32)
        m["xcls"] = np.ascontiguousarray(xs[:, 0, :]).astype(np.float32)
        in_maps.append(m)

    nc = _get_nc()
    res = run_bass_kernel_spmd(nc, in_maps, core_ids=list(range(8)))

    full = np.empty((128, 197, C), np.float32)
    for core in range(8):
        o = res.results[core]["out"]
        full[core * 16:(core + 1) * 16, 0, :] = o[NPIX:NPIX + 16]
        full[core * 16:(core + 1) * 16, 1:, :] = o[:NPIX].reshape(16, 196, C)
    return full


# revision 26
# speedup vs baseline: 1.0312x; 1.0312x over previous
"""TRN2 Bass kernel for nn_Adapter (dense_cnn): ViT adapter with two branches
  main:   h1 = xs@w1+b1 ; y = dwconv3d_3x3x3(h1)+cb ; y@w2+b2
  offset: g = xs@ow1    ; hoff = tdiff(g)+ob1 ; oc = dwconv_1x3x3(hoff)+ocb ; oc@ow2+ob2
  out = x with patch tokens += main + offset   (CLS rows pass through)

Data-parallel over 8 NeuronCores: 2 clips (16 frames) per core; adapter
weights replicated. Per-core kernel (raw bass, explicit semaphores,
fine-grained pipelining, fp8-e4m3 DoubleRow matmuls):
  - activations channel-major; x^T supplied pre-transposed (fp8) per shard
  - all three matmul stages run fp8 DoubleRow (2 contraction rows/cycle);
    depthwise convs are PSUM-accumulated diagonal DR matmuls on PE with
    taps paired within equal-dw groups (DR mid-step must be 16-aligned);
    conv outputs are scaled x16 (folded into tap weights, /16 in w2) to
    keep fp8 conv_out in the e4m3 sweet spot
  - a tunable subset of conv chunks runs on DVE (bf16 scratch accumulator,
    final cast to fp8) to balance engines
  - matmul2 token-major; DVE fuses the +x residual in f32 on evict

Self-contained: hardcodes shapes for x:[128,197,768], T=8 (asserts).
"""
import numpy as np
import ml_dtypes

import concourse.bass as bass
import concourse.mybir as mybir
from concourse.bass_utils import run_bass_kernel_spmd

F32 = mybir.dt.float32
BF16 = mybir.dt.bfloat16
F8 = mybir.dt.float8e4
AOT = mybir.AluOpType
AFT = mybir.ActivationFunctionType
DR = mybir.MatmulPerfMode.DoubleRow
BF = ml_dtypes.bfloat16
F8NP = ml_dtypes.float8_e4m3

# ---- problem constants (per core) ----
C = 768
CA = 384
T = 8
NPL = 256
CLIPS = 2
NPIX_CLIP = T * 14 * 14
NPIX = CLIPS * NPIX_CLIP
KC = C // 128
NG = CA // 128
H1PAD = NG * CLIPS * (T + 2) * NPL
GPAD = NG * CLIPS * T * NPL
GUARD = NPL
NTIL2 = (NPIX + 127) // 128
M1_CH = 392
OUT_ROWS = NPIX + 16
CVS = 16.0   # weight up-scale per stage; /CVS**3 folded into final evict

# tap (dt, dh, dw) lists grouped by dw so DR pairs share dw (step % 16 == 0)
def _pairs(taps):
    by_dw = {}
    for tp in taps:
        by_dw.setdefault(tp[2], []).append(tp)
    prs = []
    for dw in sorted(by_dw):
        grp = by_dw[dw]
        for i in range(0, len(grp) - 1, 2):
            prs.append((grp[i], grp[i + 1]))
        if len(grp) % 2:
            prs.append((grp[-1], None))
    return prs

MAIN_TAPS = [(kd - 1, kh - 1, kw - 1)
             for kd in range(3) for kh in range(3) for kw in range(3)]
OFF_TAPS = [(0, kh - 1, kw - 1) for kh in range(3) for kw in range(3)]
MAIN_PAIRS = _pairs(MAIN_TAPS)   # 15 (12 pairs + 3 singles)
OFF_PAIRS = _pairs(OFF_TAPS)     # 6 (3 pairs + 3 singles)
NPR_MAIN = len(MAIN_PAIRS)
NPR_OFF = len(OFF_PAIRS)
NPR_TOT = (NPR_MAIN + NPR_OFF) * NG   # 63

CONV_CHUNKS = [(br, g, c, tc)
               for c in range(CLIPS) for tc in range(4)
               for br in (0, 1) for g in range(NG)]


def _dve_set(n_main, n_off):
    mains = [ch for ch in CONV_CHUNKS if ch[0] == 1]
    offs = [ch for ch in CONV_CHUNKS if ch[0] == 0]
    pick = lambda lst, n: {lst[round(i * len(lst) / n) % len(lst)] for i in range(n)} if n else set()
    return pick(mains, n_main) | pick(offs, n_off)


DVE_CHUNKS = _dve_set(4, 0)


def build(debug=False, dve_chunks=None):
    dvec = DVE_CHUNKS if dve_chunks is None else dve_chunks
    nc = bass.Bass()
    xT = nc.declare_dram_parameter("xT", [C, NPIX], F8, isOutput=False)
    xtok = nc.declare_dram_parameter("xtok", [NPIX, C], F32, isOutput=False)
    xcls = nc.declare_dram_parameter("xcls", [16, C], F32, isOutput=False)
    w1c = nc.declare_dram_parameter("w1c", [128, KC // 2 * 2 * C], F8, isOutput=False)
    w2c = nc.declare_dram_parameter("w2c", [128, KC // 2 * 2 * C], F8, isOutput=False)
    diag = nc.declare_dram_parameter("diag", [128, NPR_TOT * 2 * 128], F8, isOutput=False)
    b1c = nc.declare_dram_parameter("b1c", [128, KC], F32, isOutput=False)
    cbc = nc.declare_dram_parameter("cbc", [128, KC], F32, isOutput=False)
    wtp = nc.declare_dram_parameter("wtp", [128, (27 + 9) * NG], F32, isOutput=False)
    zeros = nc.declare_dram_parameter("zeros", [128, H1PAD + 2 * GUARD], F8, isOutput=False)
    out = nc.declare_dram_parameter("out", [OUT_ROWS, C], F32, isOutput=True)
    if debug:
        dbg_h1 = nc.declare_dram_parameter("dbg_h1", [128, H1PAD + 2 * GUARD], F8, isOutput=True)
        dbg_g = nc.declare_dram_parameter("dbg_g", [128, GPAD + 2 * GUARD], F8, isOutput=True)
        dbg_cv = nc.declare_dram_parameter("dbg_cv", [128, KC * NPIX], F8, isOutput=True)

    xT_sb = nc.alloc_sbuf_tensor([128, KC * NPIX], F8)
    w1_sb = nc.alloc_sbuf_tensor([128, KC // 2 * 2 * C], F8)   # [pair][s][m]
    w2_sb = nc.alloc_sbuf_tensor([128, KC // 2 * 2 * C], F8)
    diag_sb = nc.alloc_sbuf_tensor([128, NPR_TOT * 2 * 128], F8)  # [pr][s][m]
    b1_sb = nc.alloc_sbuf_tensor([128, KC], F32)
    cb_sb = nc.alloc_sbuf_tensor([128, KC], F32)
    wt_sb = nc.alloc_sbuf_tensor([128, (27 + 9) * NG], F32)
    h1p = nc.alloc_sbuf_tensor([128, H1PAD + 2 * GUARD], F8)
    gp = nc.alloc_sbuf_tensor([128, GPAD + 2 * GUARD], F8)
    cvo = nc.alloc_sbuf_tensor([128, KC * NPIX], F8)
    scr = nc.alloc_sbuf_tensor([128, 2 * 196], BF16)    # DVE conv scratch (2 bufs)
    xtk = nc.alloc_sbuf_tensor([128, 6 * C], F32)
    ost = nc.alloc_sbuf_tensor([128, 4 * C], F32)
    ps = nc.alloc_psum_tensor([128, 4096], F32)

    M_ORDER = [3, 4, 5, 0, 1, 2]

    def h1_plane(g, c, tpad):
        return GUARD + ((g * CLIPS + c) * (T + 2) + tpad) * NPL

    def g_plane(g, c, t):
        return GUARD + ((g * CLIPS + c) * T + t) * NPL

    def sv3(buf, ext, offset, dims):
        """3D free view [part + dims] of an sbuf tensor via explicit AP."""
        return bass.AP(buf, offset, [[ext, 128]] + [list(d) for d in dims])

    XT_EXT = KC * NPIX
    W_EXT = KC // 2 * 2 * C
    DG_EXT = NPR_TOT * 2 * 128
    H1_EXT = H1PAD + 2 * GUARD
    GP_EXT = GPAD + 2 * GUARD
    CV_EXT = KC * NPIX

    # ---------- static schedules ----------
    m1_chunks = [(mi, m, j) for mi, m in enumerate(M_ORDER) for j in range(8)]
    conv_pe = [ch for ch in CONV_CHUNKS if ch not in dvec]
    conv_dve = [ch for ch in CONV_CHUNKS if ch in dvec]
    N_M1 = len(m1_chunks)
    N_CPE = len(conv_pe)
    N_CDVE = len(conv_dve)
    ACT_ALL = 2 * (N_M1 + N_CPE)
    ZERO_GP, ZERO_ALL = 32, 32   # both zero-DMAs must complete (fanout-safe)
    DIFF_ALL = ZERO_ALL + NG * CLIPS
    DVE_CONV_DONE = DIFF_ALL + N_CDVE
    DVE_EV = lambda i: DVE_CONV_DONE + i + 1
    # s_ld carries b1+cb+wtp+w2 (wait LD_WALL = all four, fanout-safe), then xtok
    LD_WALL = 64
    LD_XTOK = lambda i: 64 + 16 * (i + 1)

    def m1_thr(br, g, c, tc):
        mi = (3 + g) if br else g
        jmax = c * 4 + min(tc + 1, 3) if br else c * 4 + 3
        return 2 * (mi * 8 + jmax) + 2

    def conv_counts_thru(c_hi, tc_hi):
        npe = ndve = 0
        for ch in CONV_CHUNKS:
            br, g, c, tc = ch
            if (c, tc) > (c_hi, tc_hi):
                continue
            if ch in dvec:
                ndve += 1
            else:
                npe += 1
        return npe, ndve

    M2_THR = []
    for i in range(NTIL2):
        p_hi = (min(128 * (i + 1), NPIX) - 1) // 196
        c_hi, t_hi = divmod(p_hi, T)
        npe, ndve = conv_counts_thru(c_hi, t_hi // 2)
        M2_THR.append((2 * (N_M1 + npe), DIFF_ALL + ndve))

    with (
        nc.Block() as block,
        nc.semaphore("s_ld") as s_ld,
        nc.semaphore("s_w1") as s_w1,
        nc.semaphore("s_xt0") as s_xt0,
        nc.semaphore("s_xt1") as s_xt1,
        nc.semaphore("s_xt2") as s_xt2,
        nc.semaphore("s_dg") as s_dg,
        nc.semaphore("s_pe") as s_pe,
        nc.semaphore("s_act") as s_act,
        nc.semaphore("s_dve") as s_dve,
        nc.semaphore("s_out") as s_out,
        nc.semaphore("s_dbg") as s_dbg,
    ):
        # ================= SP: all DMA =================
        @block.sync
        def _(sync):
            sync.dma_start(out=gp[:], in_=zeros[:, :GPAD + 2 * GUARD]).then_inc(s_dve, 16)
            sync.dma_start(out=h1p[:], in_=zeros[:]).then_inc(s_dve, 16)
            sync.dma_start(out=w1_sb[:], in_=w1c[:]).then_inc(s_w1, 16)
            sync.dma_start(out=b1_sb[:], in_=b1c[:]).then_inc(s_ld, 16)
            sync.dma_start(out=cb_sb[:], in_=cbc[:]).then_inc(s_ld, 16)
            sync.dma_start(out=wt_sb[:], in_=wtp[:]).then_inc(s_ld, 16)
            for pr, sxt in enumerate((s_xt0, s_xt1, s_xt2)):
                sync.dma_start(
                    out=xT_sb[:].rearrange("p (k n) -> p k n", k=KC)[:, 2 * pr:2 * pr + 2],
                    in_=xT[:].rearrange("(k p) n -> p k n", p=128)[:, 2 * pr:2 * pr + 2],
                ).then_inc(sxt, 16)
            sync.dma_start(out=w2_sb[:], in_=w2c[:]).then_inc(s_ld, 16)
            sync.dma_start(out=diag_sb[:], in_=diag[:]).then_inc(s_dg, 16)
            sync.dma_start(out=out[NPIX:OUT_ROWS, :], in_=xcls[:]).then_inc(s_out, 16)
            for i in range(6):
                rows = min(128, NPIX - i * 128)
                sync.dma_start(out=xtk[:rows, bass.ts(i, C)],
                               in_=xtok[i * 128:i * 128 + rows, :]).then_inc(s_ld, 16)
            if debug:
                sync.wait_ge(s_act, 2 * N_M1)
                sync.wait_ge(s_dve, DIFF_ALL)
                sync.dma_start(out=dbg_h1[:], in_=h1p[:]).then_inc(s_dbg, 16)
                sync.dma_start(out=dbg_g[:], in_=gp[:]).then_inc(s_dbg, 16)
                sync.wait_ge(s_act, ACT_ALL)
                sync.wait_ge(s_dve, DVE_CONV_DONE)
                sync.dma_start(out=dbg_cv[:], in_=cvo[:]).then_inc(s_dbg, 16)
            for i in range(NTIL2):
                rows = min(128, NPIX - i * 128)
                sync.wait_ge(s_dve, DVE_EV(i))
                sync.dma_start(out=out[i * 128:i * 128 + rows, :],
                               in_=ost[:rows, bass.ts(i % 4, C)]).then_inc(s_out, 16)
                j = i + 6
                if j < NTIL2:
                    rows2 = min(128, NPIX - j * 128)
                    sync.dma_start(out=xtk[:rows2, bass.ts(j % 6, C)],
                                   in_=xtok[j * 128:j * 128 + rows2, :]).then_inc(s_ld, 16)
            if debug:
                sync.wait_ge(s_dbg, 48)

        # ================= PE =================
        @block.tensor
        def _(tensor):
            tensor.wait_ge(s_w1, 16)
            # ---- matmul1 (DR, 3 chunk-pairs), banks 0..7 rotating ----
            for q, (mi, m, j) in enumerate(m1_chunks):
                bank = q % 8
                if q >= 8:
                    tensor.wait_ge(s_act, 2 * (q - 8) + 2)
                pv = ps[:, bank * 512: bank * 512 + M1_CH]
                for pr in range(KC // 2):
                    if q == 0:
                        tensor.wait_ge((s_xt0, s_xt1, s_xt2)[pr], 16)
                    lhsT = sv3(w1_sb, W_EXT, pr * 2 * C + m * 128,
                               [(C, 2), (1, 128)])
                    rhs = sv3(xT_sb, XT_EXT, (pr * 2) * NPIX + j * M1_CH,
                              [(NPIX, 2), (1, M1_CH)])
                    mm = tensor.matmul(pv, lhsT, rhs, perf_mode=DR,
                                       start=(pr == 0), stop=(pr == KC // 2 - 1))
                mm.then_inc(s_pe, 1)
            # ---- conv (PE chunks, DR pairs), banks 4..7 rotating ----
            tensor.wait_ge(s_dg, 16)
            for qc, (br, g, c, tc) in enumerate(conv_pe):
                bank = 4 + qc % 4
                if qc >= 4:
                    tensor.wait_ge(s_act, 2 * (N_M1 + qc - 4) + 2)
                else:
                    tensor.wait_ge(s_act, 2 * (44 + qc) + 2)
                tensor.wait_ge(s_act, m1_thr(br, g, c, tc))
                if br == 0:
                    tensor.wait_ge(s_dve, ZERO_ALL + g * 2 + c + 1)
                pairs = MAIN_PAIRS if br else OFF_PAIRS
                prbase = 0 if br else NPR_MAIN * NG
                pv = ps[:, bank * 512:(bank + 1) * 512]
                for ip, (tA, tB) in enumerate(pairs):
                    dtA, dhA, dwA = tA
                    if br:
                        offA = h1_plane(g, c, 2 * tc + 1 + dtA) + dhA * 16 + dwA
                        buf, ext = h1p, H1_EXT
                    else:
                        offA = g_plane(g, c, 2 * tc + dtA) + dhA * 16 + dwA
                        buf, ext = gp, GP_EXT
                    if tB is None:
                        sstep = 16
                    else:
                        dtB, dhB, dwB = tB
                        sstep = (dtB - dtA) * 256 + (dhB - dhA) * 16
                    lhsT = sv3(diag_sb, DG_EXT, (prbase + ip * NG + g) * 256,
                               [(128, 2), (1, 128)])
                    rhs = sv3(buf, ext, offA, [(sstep, 2), (1, 512)])
                    mm = tensor.matmul(pv, lhsT, rhs, perf_mode=DR,
                                       start=(ip == 0), stop=(ip == len(pairs) - 1),
                                       skip_group_check=True)
                mm.then_inc(s_pe, 1)
            # ---- matmul2 (DR, group-pairs), psum pairs {0,1}/{2,3} ----
            for i in range(NTIL2):
                rows = min(128, NPIX - i * 128)
                ta, td = M2_THR[i]
                tensor.wait_ge(s_act, ta)
                tensor.wait_ge(s_dve, td)
                if i == 2:
                    tensor.wait_ge(s_act, ACT_ALL)   # banks 4..7 freed by conv
                if i >= 4:
                    tensor.wait_ge(s_dve, DVE_EV(i - 4))
                pv = ps[:rows, (i % 4) * 1024:(i % 4) * 1024 + 768]
                for pr in range(KC // 2):
                    lhsT = sv3(cvo, CV_EXT, (pr * 2) * NPIX + i * 128,
                               [(NPIX, 2), (1, rows)])
                    tensor.matmul(pv[:, 0:512], lhsT,
                                  sv3(w2_sb, W_EXT, pr * 2 * C, [(C, 2), (1, 512)]),
                                  perf_mode=DR,
                                  start=(pr == 0), stop=(pr == KC // 2 - 1),
                                  skip_group_check=True)
                    mm1 = tensor.matmul(pv[:, 512:768], lhsT,
                                        sv3(w2_sb, W_EXT, pr * 2 * C + 512,
                                            [(C, 2), (1, 256)]),
                                        perf_mode=DR,
                                        start=(pr == 0), stop=(pr == KC // 2 - 1),
                                        skip_group_check=True)
                mm1.then_inc(s_pe, 1)

        # ================= ACT: psum evicts =================
        @block.scalar
        def _(scalar):
            scalar.wait_ge(s_ld, LD_WALL)
            h1v = h1p[:, GUARD:GUARD + H1PAD].rearrange(
                "p (qq h w) -> p qq h w", h=16, w=16)
            gv = gp[:, GUARD:GUARD + GPAD].rearrange(
                "p (qq h w) -> p qq h w", h=16, w=16)
            seen_h1 = False
            for q, (mi, m, j) in enumerate(m1_chunks):
                bank = q % 8
                scalar.wait_ge(s_pe, q + 1)
                if q == 0:
                    scalar.wait_ge(s_dve, ZERO_GP)
                if m < 3 and not seen_h1:
                    scalar.wait_ge(s_dve, ZERO_ALL)
                    seen_h1 = True
                for pl in range(2):
                    gt = 2 * j + pl
                    c, t = divmod(gt, T)
                    src = ps[:, bank * 512 + pl * 196: bank * 512 + (pl + 1) * 196
                             ].rearrange("p (h w) -> p h w", h=14)
                    if m < 3:
                        dst = h1v[:, (m * CLIPS + c) * (T + 2) + t + 1, 1:15, 1:15]
                        bias = b1_sb[:, m:m + 1]
                    else:
                        dst = gv[:, ((m - 3) * CLIPS + c) * T + t, 1:15, 1:15]
                        bias = 0.0
                    scalar.activation(dst, src, AFT.Identity,
                                      bias=bias).then_inc(s_act, 1)
            for qc, (br, g, c, tc) in enumerate(conv_pe):
                bank = 4 + qc % 4
                scalar.wait_ge(s_pe, N_M1 + qc + 1)
                grp = g if br else 3 + g
                for pl in range(2):
                    t = 2 * tc + pl
                    src = ps[:, bank * 512 + pl * NPL + 17:
                             bank * 512 + pl * NPL + 17 + 14 * 16
                             ].rearrange("p (h w) -> p h w", w=16)[:, :, 0:14]
                    dst = cvo[:, grp * NPIX + c * NPIX_CLIP + t * 196:
                              grp * NPIX + c * NPIX_CLIP + (t + 1) * 196
                              ].rearrange("p (h w) -> p h w", h=14)
                    scalar.activation(dst, src, AFT.Identity,
                                      bias=cb_sb[:, grp:grp + 1]).then_inc(s_act, 1)

        # ================= DVE =================
        @block.vector
        def _(vector):
            vector.wait_ge(s_ld, LD_WALL)
            for g in range(NG):
                for c in range(CLIPS):
                    vector.wait_ge(s_act, 2 * (g * 8 + c * 4 + 4))
                    for t in range(T - 1, 0, -1):
                        a = g_plane(g, c, t)
                        b = g_plane(g, c, t - 1)
                        last = vector.tensor_tensor(
                            gp[:, a:a + NPL], gp[:, a:a + NPL], gp[:, b:b + NPL],
                            op=AOT.subtract)
                    z = g_plane(g, c, 0)
                    last = vector.tensor_tensor(
                        gp[:, z:z + NPL], gp[:, z:z + NPL], gp[:, z:z + NPL],
                        op=AOT.subtract)
                    ob1 = b1_sb[:, 3 + g:4 + g]
                    for t in range(T):
                        base = g_plane(g, c, t)
                        iv = gp[:, base + 17: base + 17 + 14 * 16].rearrange(
                            "p (h w) -> p h w", w=16)[:, :, 0:14]
                        last = vector.tensor_scalar(iv, iv, ob1, None, op0=AOT.add)
                    last.then_inc(s_dve, 1)
            # ---- conv chunks on DVE (bf16 scratch acc, cast to fp8 at end) ----
            for br, g, c, tc in conv_dve:
                if br == 1:
                    vector.wait_ge(s_act, m1_thr(br, g, c, tc))
                taps = MAIN_TAPS if br else OFF_TAPS
                grp = g if br else 3 + g
                wbase = (0 if br else 27 * NG)
                for pl in range(2):
                    t = 2 * tc + pl
                    acc = scr[:, pl * 196:(pl + 1) * 196].rearrange(
                        "p (h w) -> p h w", h=14)
                    for it, (dt, dh, dw) in enumerate(taps):
                        if br:
                            base = h1_plane(g, c, t + 1 + dt)
                            srcbuf = h1p
                        else:
                            base = g_plane(g, c, t + dt)
                            srcbuf = gp
                        svv = srcbuf[:, base + 17 + dh * 16 + dw:
                                     base + 17 + dh * 16 + dw + 14 * 16
                                     ].rearrange("p (h w) -> p h w", w=16)[:, :, 0:14]
                        wsc = wt_sb[:, wbase + it * NG + g: wbase + it * NG + g + 1]
                        if it == 0:
                            vector.tensor_scalar(
                                acc, svv, wsc, cb_sb[:, grp:grp + 1],
                                op0=AOT.mult, op1=AOT.add)
                        else:
                            vector.scalar_tensor_tensor(
                                acc, svv, wsc, acc, op0=AOT.mult, op1=AOT.add)
                    dst = cvo[:, grp * NPIX + c * NPIX_CLIP + t * 196:
                              grp * NPIX + c * NPIX_CLIP + (t + 1) * 196
                              ].rearrange("p (h w) -> p h w", h=14)
                    last = vector.tensor_copy(dst, acc)
                last.then_inc(s_dve, 1)
            # ---- m2 evict + residual ----
            for i in range(NTIL2):
                rows = min(128, NPIX - i * 128)
                vector.wait_ge(s_pe, N_M1 + N_CPE + i + 1)
                vector.wait_ge(s_ld, LD_XTOK(i))
                if i >= 4:
                    vector.wait_ge(s_out, 16 * (i - 2))
                vector.scalar_tensor_tensor(
                    ost[:rows, bass.ts(i % 4, C)],
                    ps[:rows, (i % 4) * 1024:(i % 4) * 1024 + 768],
                    1.0 / (CVS ** 3),
                    xtk[:rows, bass.ts(i % 6, C)],
                    op0=AOT.mult, op1=AOT.add).then_inc(s_dve, 1)

    return nc


# ---------------- host side ----------------
_NC_CACHE = {}


def _get_nc():
    if "nc" not in _NC_CACHE:
        _NC_CACHE["nc"] = build()
    return _NC_CACHE["nc"]


def _dr_pack(W):
    """[768(k), M] -> per-partition DR layout [128(ki), pair, s, M] flattened."""
    M = W.shape[1]
    out = np.zeros((128, KC // 2, 2, M), np.float32)
    for pr in range(KC // 2):
        for s in range(2):
            out[:, pr, s, :] = W[pr * 256 + s * 128: pr * 256 + (s + 1) * 128, :]
    return out.reshape(128, KC // 2 * 2 * M)


def _prep_weights(w1, b1, cw, cb, w2, b2, ow1, ob1, ocw, ocb, ow2, ob2):
    w1c = _dr_pack(np.hstack([w1, ow1]) * CVS).astype(F8NP)
    w2c = _dr_pack(np.vstack([w2, ow2]) * CVS).astype(F8NP)
    # diag DR pairs: [128(ki), pr_tot, s, 128(m)] with diagonal per s
    diag = np.zeros((128, NPR_TOT, 2, 128), np.float32)
    wtp = np.zeros((128, (27 + 9) * NG), np.float32)
    eye = np.eye(128, dtype=bool)

    def tapw(w_, tp, main):
        dt, dh, dw = tp
        if main:
            return w_[:, 0, dt + 1, dh + 1, dw + 1]
        return w_[:, 0, 0, dh + 1, dw + 1]

    for br, (pairs, w_, base) in enumerate(
            [(MAIN_PAIRS, cw, 0), (OFF_PAIRS, ocw, NPR_MAIN * NG)]):
        for ip, (tA, tB) in enumerate(pairs):
            for g in range(NG):
                pi = base + ip * NG + g
                vA = tapw(w_, tA, br == 0) * CVS
                diag[:, pi, 0, :][eye] = vA[g * 128:(g + 1) * 128]
                if tB is not None:
                    vB = tapw(w_, tB, br == 0) * CVS
                    diag[:, pi, 1, :][eye] = vB[g * 128:(g + 1) * 128]
    i = 0
    for kd in range(3):
        for kh in range(3):
            for kw in range(3):
                for g in range(NG):
                    wtp[:, i] = cw[g * 128:(g + 1) * 128, 0, kd, kh, kw] * CVS
                    i += 1
    for kh in range(3):
        for kw in range(3):
            for g in range(NG):
                wtp[:, i] = ocw[g * 128:(g + 1) * 128, 0, 0, kh, kw] * CVS
                i += 1
    b1cv = np.ascontiguousarray(
        (np.concatenate([b1, ob1]) * CVS).reshape(KC, 128).T).astype(np.float32)
    cbcv = np.ascontiguousarray(
        (np.concatenate([cb, ocb]) * CVS * CVS).reshape(KC, 128).T).astype(np.float32)
    bias2 = (b2 + ob2).astype(np.float32)
    return dict(w1c=w1c, w2c=w2c,
                diag=diag.reshape(128, NPR_TOT * 2 * 128).astype(F8NP),
                b1c=b1cv, cbc=cbcv, wtp=wtp,
                zeros=np.zeros((128, H1PAD + 2 * GUARD), F8NP)), bias2


def kernel(**inputs):
    x = np.asarray(inputs["x"], dtype=np.float32)
    Tv = int(np.asarray(inputs["T"]))
    assert Tv == T and x.shape == (128, 197, C)
    wd, bias2 = _prep_weights(
        *[np.asarray(inputs[k], dtype=np.float32) for k in
          ("w1", "b1", "cw", "cb", "w2", "b2", "ow1", "ob1", "ocw", "ocb", "ow2", "ob2")])

    in_maps = []
    for core in range(8):
        xs = x[core * 16:(core + 1) * 16]
        xpat = np.ascontiguousarray(xs[:, 1:, :]).reshape(NPIX, C)
        m = dict(wd)
        m["xT"] = np.ascontiguousarray(xpat.T).astype(F8NP)
        m["xtok"] = (xpat + bias2).astype(np.float32)
        m["xcls"] = np.ascontiguousarray(xs[:, 0, :]).astype(np.float32)
        in_maps.append(m)

    nc = _get_nc()
    res = run_bass_kernel_spmd(nc, in_maps, core_ids=list(range(8)))

    full = np.empty((128, 197, C), np.float32)
    for core in range(8):
        o = res.results[core]["out"]
        full[core * 16:(core + 1) * 16, 0, :] = o[NPIX:NPIX + 16]
        full[core * 16:(core + 1) * 16, 1:, :] = o[:NPIX].reshape(16, 196, C)
    return full


# revision 38
# speedup vs baseline: 1.2700x; 1.2317x over previous
"""TRN2 Bass kernel for nn_Adapter (dense_cnn): ViT adapter with two branches
  main:   h1 = xs@w1+b1 ; y = dwconv3d_3x3x3(h1)+cb ; y@w2+b2
  offset: g = xs@ow1    ; hoff = tdiff(g)+ob1 ; oc = dwconv_1x3x3(hoff)+ocb ; oc@ow2+ob2
  out = x with patch tokens += main + offset   (CLS rows pass through)

Data-parallel over 8 NeuronCores: 2 clips (16 frames) per core; adapter
weights replicated. Per-core kernel (raw bass, explicit semaphores,
fine-grained pipelining, fp8-e4m3 DoubleRow matmuls):
  - activations channel-major; x^T supplied pre-transposed (fp8) per shard
  - all three matmul stages run fp8 DoubleRow (2 contraction rows/cycle);
    depthwise convs are PSUM-accumulated diagonal DR matmuls on PE with
    taps paired within equal-dw groups (DR mid-step must be 16-aligned);
    conv outputs are scaled x16 (folded into tap weights, /16 in w2) to
    keep fp8 conv_out in the e4m3 sweet spot
  - a tunable subset of conv chunks runs on DVE (bf16 scratch accumulator,
    final cast to fp8) to balance engines
  - matmul2 token-major; DVE fuses the +x residual in f32 on evict

Self-contained: hardcodes shapes for x:[128,197,768], T=8 (asserts).
"""
import numpy as np
import ml_dtypes

import concourse.bass as bass
import concourse.mybir as mybir
from concourse.bass_utils import run_bass_kernel_spmd

F32 = mybir.dt.float32
BF16 = mybir.dt.bfloat16
F8 = mybir.dt.float8e4
AOT = mybir.AluOpType
AFT = mybir.ActivationFunctionType
DR = mybir.MatmulPerfMode.DoubleRow
BF = ml_dtypes.bfloat16
F8NP = ml_dtypes.float8_e4m3

# ---- problem constants (per core) ----
C = 768
CA = 384
T = 8
NPL = 256
CLIPS = 2
NPIX_CLIP = T * 14 * 14
NPIX = CLIPS * NPIX_CLIP
KC = C // 128
NG = CA // 128
H1PAD = NG * CLIPS * (T + 2) * NPL
GPAD = NG * CLIPS * T * NPL
GUARD = NPL
NTIL2 = (NPIX + 127) // 128
M1_CH = 392
OUT_ROWS = NPIX + 16
CVS = 16.0   # weight up-scale per stage; /CVS**3 folded into final evict

# tap (dt, dh, dw) lists grouped by dw so DR pairs share dw (step % 16 == 0)
def _pairs(taps):
    by_dw = {}
    for tp in taps:
        by_dw.setdefault(tp[2], []).append(tp)
    prs = []
    for dw in sorted(by_dw):
        grp = by_dw[dw]
        for i in range(0, len(grp) - 1, 2):
            prs.append((grp[i], grp[i + 1]))
        if len(grp) % 2:
            prs.append((grp[-1], None))
    return prs

MAIN_TAPS = [(kd - 1, kh - 1, kw - 1)
             for kd in range(3) for kh in range(3) for kw in range(3)]
OFF_TAPS = [(0, kh - 1, kw - 1) for kh in range(3) for kw in range(3)]
MAIN_PAIRS = _pairs(MAIN_TAPS)   # 15 (12 pairs + 3 singles)
OFF_PAIRS = _pairs(OFF_TAPS)     # 6 (3 pairs + 3 singles)
NPR_MAIN = len(MAIN_PAIRS)
NPR_OFF = len(OFF_PAIRS)
NPR_TOT = (NPR_MAIN + NPR_OFF) * NG   # 63

CONV_CHUNKS = [(br, g, c, tc)
               for c in range(CLIPS) for tc in range(4)
               for br in (0, 1) for g in range(NG)]


def _dve_set(n_main, n_off):
    mains = [ch for ch in CONV_CHUNKS if ch[0] == 1]
    offs = [ch for ch in CONV_CHUNKS if ch[0] == 0]
    return set(mains[:n_main]) | set(offs[:n_off])


DVE_CHUNKS = _dve_set(4, 0)


def build(debug=False, dve_chunks=None):
    dvec = DVE_CHUNKS if dve_chunks is None else dve_chunks
    nc = bass.Bass()
    xT = nc.declare_dram_parameter("xT", [C, NPIX], F8, isOutput=False)
    xtok = nc.declare_dram_parameter("xtok", [NPIX, C], F32, isOutput=False)
    xcls = nc.declare_dram_parameter("xcls", [16, C], F32, isOutput=False)
    w1c = nc.declare_dram_parameter("w1c", [128, KC // 2 * 2 * C], F8, isOutput=False)
    w2c = nc.declare_dram_parameter("w2c", [128, KC // 2 * 2 * C], F8, isOutput=False)
    diag = nc.declare_dram_parameter("diag", [128, NPR_TOT * 2 * 128], F8, isOutput=False)
    b1c = nc.declare_dram_parameter("b1c", [128, KC], F32, isOutput=False)
    cbc = nc.declare_dram_parameter("cbc", [128, KC], F32, isOutput=False)
    wtp = nc.declare_dram_parameter("wtp", [128, (27 + 9) * NG], F32, isOutput=False)
    zeros = nc.declare_dram_parameter("zeros", [1, 3584], F8, isOutput=False)
    out = nc.declare_dram_parameter("out", [OUT_ROWS, C], F32, isOutput=True)
    if debug:
        dbg_h1 = nc.declare_dram_parameter("dbg_h1", [128, H1PAD + 2 * GUARD], F8, isOutput=True)
        dbg_g = nc.declare_dram_parameter("dbg_g", [128, GPAD + 2 * GUARD], F8, isOutput=True)
        dbg_cv = nc.declare_dram_parameter("dbg_cv", [128, KC * NPIX], F8, isOutput=True)

    xT_sb = nc.alloc_sbuf_tensor([128, KC * NPIX], F8)
    w1_sb = nc.alloc_sbuf_tensor([128, KC // 2 * 2 * C], F8)   # [pair][s][m]
    w2_sb = nc.alloc_sbuf_tensor([128, KC // 2 * 2 * C], F8)
    diag_sb = nc.alloc_sbuf_tensor([128, NPR_TOT * 2 * 128], F8)  # [pr][s][m]
    b1_sb = nc.alloc_sbuf_tensor([128, KC], F32)
    cb_sb = nc.alloc_sbuf_tensor([128, KC], F32)
    wt_sb = nc.alloc_sbuf_tensor([128, (27 + 9) * NG], F32)
    h1p = nc.alloc_sbuf_tensor([128, H1PAD + 2 * GUARD], F8)
    gp = nc.alloc_sbuf_tensor([128, GPAD + 2 * GUARD], F8)
    cvo = nc.alloc_sbuf_tensor([128, KC * NPIX], F8)
    scr = nc.alloc_sbuf_tensor([128, 2 * 196], BF16)    # DVE conv scratch (2 bufs)
    zsb = nc.alloc_sbuf_tensor([128, 960], F8)          # zeroed tile for halo fills
    xtk = nc.alloc_sbuf_tensor([128, NTIL2 * C], F32)
    ost = nc.alloc_sbuf_tensor([128, 8 * C], F32)
    ps = nc.alloc_psum_tensor([128, 4096], F32)

    M_ORDER = [3, 4, 5, 0, 1, 2]

    def h1_plane(g, c, tpad):
        return GUARD + ((g * CLIPS + c) * (T + 2) + tpad) * NPL

    def g_plane(g, c, t):
        return GUARD + ((g * CLIPS + c) * T + t) * NPL

    def sv3(buf, ext, offset, dims):
        """3D free view [part + dims] of an sbuf tensor via explicit AP."""
        return bass.AP(buf, offset, [[ext, 128]] + [list(d) for d in dims])

    XT_EXT = KC * NPIX
    W_EXT = KC // 2 * 2 * C
    DG_EXT = NPR_TOT * 2 * 128
    H1_EXT = H1PAD + 2 * GUARD
    GP_EXT = GPAD + 2 * GUARD
    CV_EXT = KC * NPIX

    # ---------- static schedules ----------
    m1_chunks = [(mi, m, j) for mi, m in enumerate(M_ORDER) for j in range(8)]
    conv_pe = [ch for ch in CONV_CHUNKS if ch not in dvec]
    conv_dve = [ch for ch in CONV_CHUNKS if ch in dvec]
    N_M1 = len(m1_chunks)
    N_CPE = len(conv_pe)
    N_CDVE = len(conv_dve)
    ACT_ALL = 2 * (N_M1 + N_CPE)
    MS_GP, MS_ALL = 4, 8                 # DVE halo-memset incs
    DIFF_ALL = MS_ALL + NG * CLIPS
    DVE_CONV_DONE = DIFF_ALL + N_CDVE
    DVE_EVP = lambda p: DVE_CONV_DONE + p + 1    # evict-pair p (p=12: final single)
    # s_ld carries b1+cb+wtp (wait LD_WALL = all three, fanout-safe)
    LD_WALL = 48

    def m1_thr(br, g, c, tc):
        mi = (3 + g) if br else g
        jmax = c * 4 + min(tc + 1, 3) if br else c * 4 + 3
        return 2 * (mi * 8 + jmax) + 2

    def conv_counts_thru(c_hi, tc_hi):
        npe = ndve = 0
        for ch in CONV_CHUNKS:
            br, g, c, tc = ch
            if (c, tc) > (c_hi, tc_hi):
                continue
            if ch in dvec:
                ndve += 1
            else:
                npe += 1
        return npe, ndve

    M2_THR = []
    for i in range(NTIL2):
        p_hi = (min(128 * (i + 1), NPIX) - 1) // 196
        c_hi, t_hi = divmod(p_hi, T)
        npe, ndve = conv_counts_thru(c_hi, t_hi // 2)
        M2_THR.append((2 * (N_M1 + npe), DIFF_ALL + ndve))

    from contextlib import ExitStack
    _sems = ExitStack()
    xk = [_sems.enter_context(nc.semaphore(f"s_xk{i}")) for i in range(8)]
    ot = [_sems.enter_context(nc.semaphore(f"s_ot{i}")) for i in range(8)]
    with (
        _sems,
        nc.Block() as block,
        nc.semaphore("s_ld") as s_ld,
        nc.semaphore("s_w1") as s_w1,
        nc.semaphore("s_xt0") as s_xt0,
        nc.semaphore("s_xt1") as s_xt1,
        nc.semaphore("s_xt2") as s_xt2,
        nc.semaphore("s_dg") as s_dg,
        nc.semaphore("s_w2") as s_w2,
        nc.semaphore("s_z1") as s_z1,
        nc.semaphore("s_z2") as s_z2,
        nc.semaphore("s_pe") as s_pe,
        nc.semaphore("s_act") as s_act,
        nc.semaphore("s_dve") as s_dve,
        nc.semaphore("s_out") as s_out,
        nc.semaphore("s_dbg") as s_dbg,
    ):
        # ================= SP: all DMA =================
        @block.sync
        def _(sync):
            sync.dma_start(out=w1_sb[:], in_=w1c[:]).then_inc(s_w1, 16)
            sync.dma_start(
                out=xT_sb[:].rearrange("p (k n) -> p k n", k=KC)[:, :, 0:1568],
                in_=xT[:].rearrange("(k p) n -> p k n", p=128)[:, :, 0:1568],
            ).then_inc(s_xt0, 16)
            # zero-fill: gp guards (2 runs) + h1p t-halo/guard runs (7x512)
            sync.dma_start(
                out=bass.AP(gp, 0, [[GP_EXT, 128], [GUARD + GPAD, 2], [1, GUARD]]),
                in_=bass.AP(zeros, 0, [[0, 128], [GUARD, 2], [1, GUARD]]),
            ).then_inc(s_z1, 16)
            sync.dma_start(out=zsb[:],
                           in_=bass.AP(zeros, 0, [[0, 128], [1, 960]])).then_inc(s_z1, 16)
            sync.dma_start(out=b1_sb[:], in_=b1c[:]).then_inc(s_ld, 16)
            sync.dma_start(out=cb_sb[:], in_=cbc[:]).then_inc(s_ld, 16)
            sync.dma_start(out=wt_sb[:], in_=wtp[:]).then_inc(s_ld, 16)
            sync.dma_start(
                out=bass.AP(h1p, 0, [[H1_EXT, 128], [2560, 7], [1, 512]]),
                in_=bass.AP(zeros, 0, [[0, 128], [512, 7], [1, 512]]),
            ).then_inc(s_z2, 16)
            sync.dma_start(
                out=xT_sb[:].rearrange("p (k n) -> p k n", k=KC)[:, :, 1568:3136],
                in_=xT[:].rearrange("(k p) n -> p k n", p=128)[:, :, 1568:3136],
            ).then_inc(s_xt1, 16)
            sync.dma_start(out=w2_sb[:], in_=w2c[:]).then_inc(s_w2, 16)
            sync.dma_start(out=diag_sb[:], in_=diag[:]).then_inc(s_dg, 16)
            sync.dma_start(out=out[NPIX:OUT_ROWS, :], in_=xcls[:]).then_inc(s_out, 16)
            for pj in range(12):     # all xtok pair-loads up front
                j = 2 * pj
                sync.dma_start(
                    out=xtk[:, j * C:(j + 2) * C].rearrange("p (b c) -> p b c", b=2),
                    in_=xtok[j * 128:(j + 2) * 128, :].rearrange("(b r) c -> r b c", b=2),
                ).then_inc(xk[pj % 8], 16)
            sync.dma_start(out=xtk[:64, bass.ts(24, C)],
                           in_=xtok[24 * 128:NPIX, :]).then_inc(xk[12 % 8], 16)
            if debug:
                sync.wait_ge(s_act, 2 * N_M1)
                sync.wait_ge(s_dve, DIFF_ALL)
                sync.dma_start(out=dbg_h1[:], in_=h1p[:]).then_inc(s_dbg, 16)
                sync.dma_start(out=dbg_g[:], in_=gp[:]).then_inc(s_dbg, 16)
                sync.wait_ge(s_act, ACT_ALL)
                sync.wait_ge(s_dve, DVE_CONV_DONE)
                sync.dma_start(out=dbg_cv[:], in_=cvo[:]).then_inc(s_dbg, 16)
            for p in range(NTIL2 // 2):          # 12 pairs
                i = 2 * p
                sync.wait_ge(s_dve, DVE_EVP(p))
                sync.dma_start(
                    out=out[i * 128:(i + 2) * 128, :].rearrange("(b r) c -> r b c", b=2),
                    in_=ost[:, (p % 4) * 2 * C:((p % 4) * 2 + 2) * C].rearrange("p (b c) -> p b c", b=2),
                ).then_inc(ot[p % 8], 16)
            sync.wait_ge(s_dve, DVE_EVP(12))
            sync.dma_start(out=out[24 * 128:NPIX, :],
                           in_=ost[:64, bass.ts((12 % 4) * 2, C)]).then_inc(ot[12 % 8], 16)
            if debug:
                sync.wait_ge(s_dbg, 48)

        # ================= PE =================
        @block.tensor
        def _(tensor):
            tensor.wait_ge(s_w1, 16)
            # ---- matmul1 (DR, 3 chunk-pairs), banks 0..7 rotating ----
            for q, (mi, m, j) in enumerate(m1_chunks):
                bank = q % 8
                if q >= 8:
                    tensor.wait_ge(s_act, 2 * (q - 8) + 2)
                pv = ps[:, bank * 512: bank * 512 + M1_CH]
                if q == 0:
                    tensor.wait_ge(s_xt0, 16)
                elif q == 4:
                    tensor.wait_ge(s_xt1, 16)
                for pr in range(KC // 2):
                    lhsT = sv3(w1_sb, W_EXT, pr * 2 * C + m * 128,
                               [(C, 2), (1, 128)])
                    rhs = sv3(xT_sb, XT_EXT, (pr * 2) * NPIX + j * M1_CH,
                              [(NPIX, 2), (1, M1_CH)])
                    mm = tensor.matmul(pv, lhsT, rhs, perf_mode=DR,
                                       start=(pr == 0), stop=(pr == KC // 2 - 1))
                mm.then_inc(s_pe, 1)
            # ---- conv (PE chunks, DR pairs), banks 4..7 rotating ----
            tensor.wait_ge(s_dg, 16)
            for qc, (br, g, c, tc) in enumerate(conv_pe):
                bank = 4 + qc % 4
                if qc >= 4:
                    tensor.wait_ge(s_act, 2 * (N_M1 + qc - 4) + 2)
                else:
                    tensor.wait_ge(s_act, 2 * (44 + qc) + 2)
                tensor.wait_ge(s_act, m1_thr(br, g, c, tc))
                if br == 0:
                    tensor.wait_ge(s_dve, MS_ALL + g * 2 + c + 1)
                pairs = MAIN_PAIRS if br else OFF_PAIRS
                prbase = 0 if br else NPR_MAIN * NG
                pv = ps[:, bank * 512:(bank + 1) * 512]
                for ip, (tA, tB) in enumerate(pairs):
                    dtA, dhA, dwA = tA
                    if br:
                        offA = h1_plane(g, c, 2 * tc + 1 + dtA) + dhA * 16 + dwA
                        buf, ext = h1p, H1_EXT
                    else:
                        offA = g_plane(g, c, 2 * tc + dtA) + dhA * 16 + dwA
                        buf, ext = gp, GP_EXT
                    if tB is None:
                        sstep = 16
                    else:
                        dtB, dhB, dwB = tB
                        sstep = (dtB - dtA) * 256 + (dhB - dhA) * 16
                    lhsT = sv3(diag_sb, DG_EXT, (prbase + ip * NG + g) * 256,
                               [(128, 2), (1, 128)])
                    rhs = sv3(buf, ext, offA, [(sstep, 2), (1, 512)])
                    mm = tensor.matmul(pv, lhsT, rhs, perf_mode=DR,
                                       start=(ip == 0), stop=(ip == len(pairs) - 1),
                                       skip_group_check=True)
                mm.then_inc(s_pe, 1)
            # ---- matmul2 (DR, group-pairs), psum pairs {0,1}/{2,3} ----
            tensor.wait_ge(s_w2, 16)
            for i in range(NTIL2):
                rows = min(128, NPIX - i * 128)
                ta, td = M2_THR[i]
                tensor.wait_ge(s_act, ta)
                tensor.wait_ge(s_dve, td)
                if i == 2:
                    tensor.wait_ge(s_act, ACT_ALL)   # banks 4..7 freed by conv
                if i >= 4:
                    tensor.wait_ge(s_dve, DVE_EVP((i - 4) // 2))
                pv = ps[:rows, (i % 4) * 1024:(i % 4) * 1024 + 768]
                for pr in range(KC // 2):
                    lhsT = sv3(cvo, CV_EXT, (pr * 2) * NPIX + i * 128,
                               [(NPIX, 2), (1, rows)])
                    tensor.matmul(pv[:, 0:512], lhsT,
                                  sv3(w2_sb, W_EXT, pr * 2 * C, [(C, 2), (1, 512)]),
                                  perf_mode=DR,
                                  start=(pr == 0), stop=(pr == KC // 2 - 1),
                                  skip_group_check=True)
                    mm1 = tensor.matmul(pv[:, 512:768], lhsT,
                                        sv3(w2_sb, W_EXT, pr * 2 * C + 512,
                                            [(C, 2), (1, 256)]),
                                        perf_mode=DR,
                                        start=(pr == 0), stop=(pr == KC // 2 - 1),
                                        skip_group_check=True)
                mm1.then_inc(s_pe, 1)

        # ================= ACT: psum evicts =================
        @block.scalar
        def _(scalar):
            scalar.wait_ge(s_ld, LD_WALL)
            h1v = h1p[:, GUARD:GUARD + H1PAD].rearrange(
                "p (qq h w) -> p qq h w", h=16, w=16)
            gv = gp[:, GUARD:GUARD + GPAD].rearrange(
                "p (qq h w) -> p qq h w", h=16, w=16)
            seen_h1 = False
            for q, (mi, m, j) in enumerate(m1_chunks):
                bank = q % 8
                scalar.wait_ge(s_pe, q + 1)
                if q == 0:
                    scalar.wait_ge(s_z1, 32)
                    scalar.wait_ge(s_dve, MS_GP)
                if m < 3 and not seen_h1:
                    scalar.wait_ge(s_z2, 16)
                    scalar.wait_ge(s_dve, MS_ALL)
                    seen_h1 = True
                for pl in range(2):
                    gt = 2 * j + pl
                    c, t = divmod(gt, T)
                    src = ps[:, bank * 512 + pl * 196: bank * 512 + (pl + 1) * 196
                             ].rearrange("p (h w) -> p h w", h=14)
                    if m < 3:
                        dst = h1v[:, (m * CLIPS + c) * (T + 2) + t + 1, 1:15, 1:15]
                        bias = b1_sb[:, m:m + 1]
                    else:
                        dst = gv[:, ((m - 3) * CLIPS + c) * T + t, 1:15, 1:15]
                        bias = 0.0
                    scalar.activation(dst, src, AFT.Identity,
                                      bias=bias).then_inc(s_act, 1)
            for qc, (br, g, c, tc) in enumerate(conv_pe):
                bank = 4 + qc % 4
                scalar.wait_ge(s_pe, N_M1 + qc + 1)
                grp = g if br else 3 + g
                for pl in range(2):
                    t = 2 * tc + pl
                    src = ps[:, bank * 512 + pl * NPL + 17:
                             bank * 512 + pl * NPL + 17 + 14 * 16
                             ].rearrange("p (h w) -> p h w", w=16)[:, :, 0:14]
                    dst = cvo[:, grp * NPIX + c * NPIX_CLIP + t * 196:
                              grp * NPIX + c * NPIX_CLIP + (t + 1) * 196
                              ].rearrange("p (h w) -> p h w", h=14)
                    scalar.activation(dst, src, AFT.Identity,
                                      bias=cb_sb[:, grp:grp + 1]).then_inc(s_act, 1)

        # ================= DVE =================
        @block.vector
        def _(vector):
            # halo zero-fills: rows 0/15 and cols 0/15 of every padded plane
            # (tensor_copy from a DMA-zeroed tile; DVE memset is unreliable)
            vector.wait_ge(s_z1, 32)       # gp guards + zsb
            for buf, ext, npl_ in ((gp, GP_EXT, 48), (h1p, H1_EXT, 60)):
                for off, dims in (
                    (GUARD, [[256, npl_], [1, 16]]),           # row 0
                    (GUARD + 240, [[256, npl_], [1, 16]]),     # row 15
                    (GUARD, [[256, npl_], [16, 16]]),          # col 0
                    (GUARD + 15, [[256, npl_], [16, 16]]),     # col 15
                ):
                    vector.tensor_copy(
                        bass.AP(buf, off, [[ext, 128]] + dims),
                        bass.AP(zsb, 0, [[960, 128], [16, npl_], [1, 16]]),
                    ).then_inc(s_dve, 1)
            vector.wait_ge(s_ld, LD_WALL)
            for g in range(NG):
                for c in range(CLIPS):
                    vector.wait_ge(s_act, 2 * (g * 8 + c * 4 + 4))
                    for t in range(T - 1, 0, -1):
                        a = g_plane(g, c, t)
                        b = g_plane(g, c, t - 1)
                        last = vector.tensor_tensor(
                            gp[:, a:a + NPL], gp[:, a:a + NPL], gp[:, b:b + NPL],
                            op=AOT.subtract)
                    z = g_plane(g, c, 0)
                    last = vector.tensor_tensor(
                        gp[:, z:z + NPL], gp[:, z:z + NPL], gp[:, z:z + NPL],
                        op=AOT.subtract)
                    ob1 = b1_sb[:, 3 + g:4 + g]
                    for t in range(T):
                        base = g_plane(g, c, t)
                        iv = gp[:, base + 17: base + 17 + 14 * 16].rearrange(
                            "p (h w) -> p h w", w=16)[:, :, 0:14]
                        last = vector.tensor_scalar(iv, iv, ob1, None, op0=AOT.add)
                    last.then_inc(s_dve, 1)
            # ---- conv chunks on DVE (bf16 scratch acc, cast to fp8 at end) ----
            for br, g, c, tc in conv_dve:
                if br == 1:
                    vector.wait_ge(s_act, m1_thr(br, g, c, tc))
                taps = MAIN_TAPS if br else OFF_TAPS
                grp = g if br else 3 + g
                wbase = (0 if br else 27 * NG)
                for pl in range(2):
                    t = 2 * tc + pl
                    acc = scr[:, pl * 196:(pl + 1) * 196].rearrange(
                        "p (h w) -> p h w", h=14)
                    for it, (dt, dh, dw) in enumerate(taps):
                        if br:
                            base = h1_plane(g, c, t + 1 + dt)
                            srcbuf = h1p
                        else:
                            base = g_plane(g, c, t + dt)
                            srcbuf = gp
                        svv = srcbuf[:, base + 17 + dh * 16 + dw:
                                     base + 17 + dh * 16 + dw + 14 * 16
                                     ].rearrange("p (h w) -> p h w", w=16)[:, :, 0:14]
                        wsc = wt_sb[:, wbase + it * NG + g: wbase + it * NG + g + 1]
                        if it == 0:
                            vector.tensor_scalar(
                                acc, svv, wsc, cb_sb[:, grp:grp + 1],
                                op0=AOT.mult, op1=AOT.add)
                        else:
                            vector.scalar_tensor_tensor(
                                acc, svv, wsc, acc, op0=AOT.mult, op1=AOT.add)
                    dst = cvo[:, grp * NPIX + c * NPIX_CLIP + t * 196:
                              grp * NPIX + c * NPIX_CLIP + (t + 1) * 196
                              ].rearrange("p (h w) -> p h w", h=14)
                    last = vector.tensor_copy(dst, acc)
                last.then_inc(s_dve, 1)
            # ---- m2 evict + residual (paired: 2 tiles per op) ----
            for p in range(NTIL2 // 2 + 1):
                i = 2 * p
                if p == 12:      # final single tile, 64 rows
                    vector.wait_ge(s_pe, N_M1 + N_CPE + 25)
                    vector.wait_ge(xk[12 % 8], 16 * (12 // 8 + 1))
                    jj = (24 - 4) // 2
                    vector.wait_ge(ot[jj % 8], 16 * (jj // 8 + 1))
                    vector.scalar_tensor_tensor(
                        ost[:64, bass.ts((12 % 4) * 2, C)],
                        ps[:64, (24 % 4) * 1024:(24 % 4) * 1024 + 768],
                        1.0 / (CVS ** 3),
                        xtk[:64, bass.ts(24, C)],
                        op0=AOT.mult, op1=AOT.add).then_inc(s_dve, 1)
                    break
                vector.wait_ge(s_pe, N_M1 + N_CPE + i + 2)
                vector.wait_ge(xk[p % 8], 16 * (p // 8 + 1))
                if p >= 4:
                    jj = p - 4
                    vector.wait_ge(ot[jj % 8], 16 * (jj // 8 + 1))
                vector.scalar_tensor_tensor(
                    sv3(ost, 8 * C, (p % 4) * 2 * C, [(C, 2), (1, C)]),
                    sv3(ps, 4096, (i % 4) * 1024, [(1024, 2), (1, 768)]),
                    1.0 / (CVS ** 3),
                    sv3(xtk, NTIL2 * C, i * C, [(C, 2), (1, C)]),
                    op0=AOT.mult, op1=AOT.add).then_inc(s_dve, 1)

    return nc


# ---------------- host side ----------------
_NC_CACHE = {}


def _get_nc():
    if "nc" not in _NC_CACHE:
        _NC_CACHE["nc"] = build()
    return _NC_CACHE["nc"]


def _dr_pack(W):
    """[768(k), M] -> per-partition DR layout [128(ki), pair, s, M] flattened."""
    M = W.shape[1]
    out = np.zeros((128, KC // 2, 2, M), np.float32)
    for pr in range(KC // 2):
        for s in range(2):
            out[:, pr, s, :] = W[pr * 256 + s * 128: pr * 256 + (s + 1) * 128, :]
    return out.reshape(128, KC // 2 * 2 * M)


def _prep_weights(w1, b1, cw, cb, w2, b2, ow1, ob1, ocw, ocb, ow2, ob2):
    w1c = _dr_pack(np.hstack([w1, ow1]) * CVS).astype(F8NP)
    w2c = _dr_pack(np.vstack([w2, ow2]) * CVS).astype(F8NP)
    # diag DR pairs: [128(ki), pr_tot, s, 128(m)] with diagonal per s
    diag = np.zeros((128, NPR_TOT, 2, 128), np.float32)
    wtp = np.zeros((128, (27 + 9) * NG), np.float32)
    eye = np.eye(128, dtype=bool)

    def tapw(w_, tp, main):
        dt, dh, dw = tp
        if main:
            return w_[:, 0, dt + 1, dh + 1, dw + 1]
        return w_[:, 0, 0, dh + 1, dw + 1]

    for br, (pairs, w_, base) in enumerate(
            [(MAIN_PAIRS, cw, 0), (OFF_PAIRS, ocw, NPR_MAIN * NG)]):
        for ip, (tA, tB) in enumerate(pairs):
            for g in range(NG):
                pi = base + ip * NG + g
                vA = tapw(w_, tA, br == 0) * CVS
                diag[:, pi, 0, :][eye] = vA[g * 128:(g + 1) * 128]
                if tB is not None:
                    vB = tapw(w_, tB, br == 0) * CVS
                    diag[:, pi, 1, :][eye] = vB[g * 128:(g + 1) * 128]
    i = 0
    for kd in range(3):
        for kh in range(3):
            for kw in range(3):
                for g in range(NG):
                    wtp[:, i] = cw[g * 128:(g + 1) * 128, 0, kd, kh, kw] * CVS
                    i += 1
    for kh in range(3):
        for kw in range(3):
            for g in range(NG):
                wtp[:, i] = ocw[g * 128:(g + 1) * 128, 0, 0, kh, kw] * CVS
                i += 1
    b1cv = np.ascontiguousarray(
        (np.concatenate([b1, ob1]) * CVS).reshape(KC, 128).T).astype(np.float32)
    cbcv = np.ascontiguousarray(
        (np.concatenate([cb, ocb]) * CVS * CVS).reshape(KC, 128).T).astype(np.float32)
    bias2 = (b2 + ob2).astype(np.float32)
    return dict(w1c=w1c, w2c=w2c,
                diag=diag.reshape(128, NPR_TOT * 2 * 128).astype(F8NP),
                b1c=b1cv, cbc=cbcv, wtp=wtp,
                zeros=np.zeros((1, 3584), F8NP)), bias2


def kernel(**inputs):
    x = np.asarray(inputs["x"], dtype=np.float32)
    Tv = int(np.asarray(inputs["T"]))
    assert Tv == T and x.shape == (128, 197, C)
    wd, bias2 = _prep_weights(
        *[np.asarray(inputs[k], dtype=np.float32) for k in
          ("w1", "b1", "cw", "cb", "w2", "b2", "ow1", "ob1", "ocw", "ocb", "ow2", "ob2")])

    in_maps = []
    for core in range(8):
        xs = x[core * 16:(core + 1) * 16]
        xpat = np.ascontiguousarray(xs[:, 1:, :]).reshape(NPIX, C)
        m = dict(wd)
        m["xT"] = np.ascontiguousarray(xpat.T).astype(F8NP)
        m["xtok"] = (xpat + bias2).astype(np.float32)
        m["xcls"] = np.ascontiguousarray(xs[:, 0, :]).astype(np.float32)
        in_maps.append(m)

    nc = _get_nc()
    res = run_bass_kernel_spmd(nc, in_maps, core_ids=list(range(8)))

    full = np.empty((128, 197, C), np.float32)
    for core in range(8):
        o = res.results[core]["out"]
        full[core * 16:(core + 1) * 16, 0, :] = o[NPIX:NPIX + 16]
        full[core * 16:(core + 1) * 16, 1:, :] = o[:NPIX].reshape(16, 196, C)
    return full
